# revision 15
# baseline (speedup 1.0000x reference)
# Bass/Tile Trainium2 kernel for batched multi-head causal self-attention.
#
# Problem: x[B=2,T=2048,C=1024], 16 heads (hd=64), causal softmax attention,
# output projection. Full (unsharded) inputs in, full output out.
#
# Sharding (Megatron-style): 8 cores = 2 batch groups x 4 head groups.
# Core i handles batch b = i // 4 and heads [4*(i%4) : 4*(i%4)+4).
# Each core computes Q/K/V projections for its 4 heads, causal attention,
# and a partial output projection (contribution of its heads).  The host
# sums the 4 partials per batch (the Megatron all-reduce) and adds bias.
#
# Schedule: a software pipeline interleaved at ~0.5us granularity.
# Attention is ACT(exp)-bound, so QKV matmuls for t-block tb+1 and the
# output-projection matmuls for q-block qb-1 are emitted as "filler"
# chunks between the S and P@V matmuls of q-block qb, keeping the PE
# busy while the Scalar engine chews through exp.  All DMA is routed on
# the Sync/GpSimd/Vector queues so the Scalar queue runs ACTIVATE only.
#
# On-device layout notes:
#   - Everything is kept "transposed" (feature dim on partitions):
#     xT [C, T], QT/KT [64, T] per head.  Heads come in pairs packed on
#     the 128 partitions (even head at [0:64], odd head at [64:128]); the
#     K=64 S^T matmuls of a pair use explicit tile_position row groups so
#     they run concurrently on disjoint PE quadrants.
#   - V is stored [T, 64] per head augmented with a ones column (V') so
#     the P@V matmul also produces the softmax denominator (row 64).
#   - Softmax runs without max-subtraction (scores are bounded ~|10|, exp
#     is safe in fp32), so no partition-dim reductions are ever needed.
#   - S^T tiles for a (head-pair, 2 k-tiles) group live in one 4-bank
#     PSUM tile [128, 2048] = (ktile, slot, q); exp is ONE activation
#     instruction over the whole group.
#   - Causal masking: k-tiles strictly above the diagonal are skipped;
#     tiles crossing the diagonal get a triangular mask multiply (one
#     [128,2,128] DVE op per k-tile covering both slots) and a
#     column-restricted P@V matmul.
#   - QKV+attention internals are bf16; the projection also runs in bf16
#     (O^T normalized into bf16, Wp bf16) and partials are DMA'd out in
#     bf16; the host accumulates in fp32.

import numpy as np
from collections import deque

import concourse.bass as bass
import concourse.tile as tile
from concourse import bacc, mybir
from concourse import bass_utils

F32 = mybir.dt.float32
BF16 = mybir.dt.bfloat16
ATT_DT = BF16

B, T, C, H = 2, 2048, 1024, 16
HD = C // H            # 64 head dim
NCORES = 8
HPC = 4                # heads per core
DSEL = HPC * HD        # 256 feature dims per core
NTT = T // 128         # 16 t-tiles of 128
NCC = C // 128         # 8 c-chunks of 128
NQB = T // 512         # 4 q-blocks of 512


def build_program():
    nc = bacc.Bacc("TRN2", target_bir_lowering=False, debug=False)

    xT = nc.dram_tensor("xT", [128, NQB, NCC, 512], BF16, kind="ExternalInput").ap()
    wqT = nc.dram_tensor("wqT", [128, NCC * DSEL], BF16, kind="ExternalInput").ap()
    wkT = nc.dram_tensor("wkT", [128, NCC * DSEL], BF16, kind="ExternalInput").ap()
    wvT = nc.dram_tensor("wvT", [128, NCC * DSEL], BF16, kind="ExternalInput").ap()
    wpT = nc.dram_tensor("wpT", [128, 2 * C], BF16, kind="ExternalInput").ap()
    maskd = nc.dram_tensor("maskd", [128, 256], ATT_DT, kind="ExternalInput").ap()
    out_p = nc.dram_tensor("out_p", [T, C], BF16, kind="ExternalOutput").ap()

    scale = 1.0 / float(np.sqrt(HD))

    with tile.TileContext(nc) as tc:
        with (
            tc.tile_pool(name="consts", bufs=1) as consts,
            tc.tile_pool(name="persist", bufs=1) as persist,
            tc.tile_pool(name="pt", bufs=3) as ptpool,
            tc.tile_pool(name="psn", bufs=6) as psn,
            tc.tile_pool(name="nsm", bufs=4) as nsm,
            tc.tile_pool(name="outst", bufs=4) as outst,
            tc.tile_pool(name="sgp", bufs=1, space="PSUM") as sgp,
            tc.tile_pool(name="pop", bufs=1, space="PSUM") as pop,
            tc.tile_pool(name="gpp", bufs=2, space="PSUM") as gpp,
        ):
            # ---- ACT exp table pre-warm (one-time ~2.7us table DMA) ----
            wrm_in = consts.tile([1, 16], F32, tag="wrm_in")
            wrm_out = consts.tile([1, 16], BF16, tag="wrm_out")
            nc.vector.memset(wrm_in[:], 0.0)
            nc.scalar.activation(
                out=wrm_out[:], in_=wrm_in[:],
                func=mybir.ActivationFunctionType.Exp, scale=1.0,
            )

            # ---- constants / weights ----------------------------------
            wq_sb = consts.tile([128, NCC, DSEL], BF16, tag="wq")
            wk_sb = consts.tile([128, NCC, DSEL], BF16, tag="wk")
            wv_sb = consts.tile([128, NCC, DSEL], BF16, tag="wv")
            wp_sb = consts.tile([128, 2, C], BF16, tag="wp")
            mk_sb = consts.tile([128, 256], ATT_DT, tag="mk")
            nc.sync.dma_start(out=mk_sb[:], in_=maskd)

            # persistent activations: QT/KT/OT head pairs packed on
            # partitions ([0:64] even slot, [64:128] odd slot), free = t
            qt_sb = persist.tile([128, 2, T], ATT_DT, tag="qt")
            kt_sb = persist.tile([128, 2, T], ATT_DT, tag="kt")
            ot_sb = persist.tile([128, 2, T], ATT_DT, tag="ot")
            # V' per k-tile: 4 heads x (64 V cols + 1 ones col)
            v_sb = persist.tile([128, NTT, HPC * (HD + 1)], ATT_DT, tag="v")
            xt_all = persist.tile([128, NQB, NCC, 512], BF16, tag="xt")
            den = [persist.tile([2, NQB, 512], F32, tag=f"den{hp}", name=f"den{hp}")
                   for hp in range(2)]
            rec = [persist.tile([2, NQB, 512], F32, tag=f"rec{hp}", name=f"rec{hp}")
                   for hp in range(2)]

            ones_sb = consts.tile([128, NTT], F32, tag="ones")
            nc.vector.memset(ones_sb[:], 1.0)
            for h in range(HPC):
                nc.vector.tensor_copy(
                    out=v_sb[:, :, h * 65 + 64 : h * 65 + 65],
                    in_=ones_sb[:].rearrange("p (t o) -> p t o", o=1),
                )

            # ---- input DMAs: few, large, queue-parallel ---------------
            # x is staged in DRAM as [128, tb, cc, 512] so each t-block is
            # one contiguous 8KB-per-partition DMA; t-blocks 1-3 ride the
            # Scalar queue (idle until attention starts).
            nc.sync.dma_start(
                out=wq_sb[:].rearrange("p c d -> p (c d)"), in_=wqT[:])
            nc.sync.dma_start(
                out=xt_all[:, 0].rearrange("p c t -> p (c t)"),
                in_=xT[:, 0].rearrange("p c t -> p (c t)"))
            nc.sync.dma_start(
                out=wk_sb[:].rearrange("p c d -> p (c d)"), in_=wkT[:])
            nc.sync.dma_start(
                out=wv_sb[:].rearrange("p c d -> p (c d)"), in_=wvT[:])
            for tb in range(1, 4):
                nc.scalar.dma_start(
                    out=xt_all[:, tb].rearrange("p c t -> p (c t)"),
                    in_=xT[:, tb].rearrange("p c t -> p (c t)"))
            nc.sync.dma_start(
                out=wp_sb[:].rearrange("p h c -> p (h c)"), in_=wpT[:])

            # ---- filler generators (PE work interleaved into attention)
            def gen_qk(tb):
                ts = slice(tb * 512, tb * 512 + 512)
                # Q then K: one 512-wide matmul chain per head-pair
                for wsb, dst in ((wq_sb, qt_sb), (wk_sb, kt_sb)):
                    for pr in range(2):
                        acc = gpp.tile([128, 512], F32, tag="gp", name="acc")
                        for cc in range(NCC):
                            nc.tensor.matmul(
                                acc[:],
                                wsb[:, cc, pr * 128 : pr * 128 + 128],
                                xt_all[:, tb, cc, :],
                                start=(cc == 0), stop=(cc == NCC - 1),
                            )
                            if cc % 3 == 2:
                                yield
                        nc.vector.tensor_copy(out=dst[:, pr, ts], in_=acc[:])
                        yield

            def gen_v(tb):
                # V: [t, d] layout, two 256-t halves of the block
                for half in range(2):
                    accv = gpp.tile([128, 512], F32, tag="gp", name="accv")
                    for cc in range(NCC):
                        for tt in range(2):
                            tl = half * 256 + tt * 128
                            nc.tensor.matmul(
                                accv[:, tt * 256 : tt * 256 + 256],
                                xt_all[:, tb, cc, tl : tl + 128],
                                wv_sb[:, cc, :],
                                start=(cc == 0 and tt == 0),
                                stop=(cc == NCC - 1 and tt == 1),
                            )
                        if cc % 3 == 2:
                            yield
                    t4 = tb * 4 + half * 2
                    pv3 = accv[:].rearrange("p (tt d) -> p tt d", tt=2)
                    for h in range(HPC):
                        nc.vector.tensor_copy(
                            out=v_sb[:, t4 : t4 + 2, h * 65 : h * 65 + 64],
                            in_=pv3[:, :, h * 64 : h * 64 + 64],
                        )
                    yield

            def gen_proj(qb):
                for tt in range(4 * qb, 4 * qb + 4):
                    tloc = slice(tt * 128, tt * 128 + 128)
                    pc0 = gpp.tile([128, 512], F32, tag="gp", name="pc0")
                    pc1 = gpp.tile([128, 512], F32, tag="gp", name="pc1")
                    for hpp in range(2):
                        for cb, pc in enumerate((pc0, pc1)):
                            nc.tensor.matmul(
                                pc[:],
                                ot_sb[:, hpp, tloc],
                                wp_sb[:, hpp, cb * 512 : cb * 512 + 512],
                                start=(hpp == 0), stop=(hpp == 1),
                            )
                        yield
                    ob = outst.tile([128, 1024], BF16, tag="ob", name="ob")
                    nc.vector.tensor_copy(out=ob[:, 0:512], in_=pc0[:])
                    nc.scalar.copy(ob[:, 512:1024], pc1[:])
                    nc.sync.dma_start(
                        out=out_p[tt * 128 : tt * 128 + 128, :], in_=ob[:])
                    yield

            fillers = deque()

            def pump(n):
                while n > 0 and fillers:
                    try:
                        next(fillers[0])
                        n -= 1
                    except StopIteration:
                        fillers.popleft()

            def drain(gen):
                while gen in fillers:
                    pump(1)

            # ---- attention per (q-block, head-pair) -------------------
            def emit_attn(qb):
                qs = slice(qb * 512, qb * 512 + 512)
                n_kt = 4 * (qb + 1)
                for hp in range(2):
                    po = pop.tile([128, 1024], F32, tag="po", name="po")
                    prev_pv = None
                    for g in range(n_kt // 2):
                        sg = sgp.tile([128, 2048], F32, tag="sg", name="sg")
                        pt = ptpool.tile([128, 2048], ATT_DT, tag="pt", name="pt")
                        # S^T: 4 matmuls, row-split pairs run concurrently
                        for ktl in range(2):
                            kt = 2 * g + ktl
                            for s in range(2):
                                psl = slice(64 * s, 64 * s + 64)
                                nc.tensor.matmul(
                                    sg[:, ktl * 1024 + s * 512 : ktl * 1024 + s * 512 + 512],
                                    kt_sb[psl, hp, kt * 128 : kt * 128 + 128],
                                    qt_sb[psl, hp, qs],
                                    start=True, stop=True,
                                    tile_position=(64 * s, 0),
                                )
                        # exp over the whole group (both k-tiles, both slots)
                        nc.scalar.activation(
                            out=pt[:], in_=sg[:],
                            func=mybir.ActivationFunctionType.Exp,
                            scale=scale,
                        )
                        # triangular mask on diagonal blocks (both slots/op)
                        pt4 = pt[:].rearrange("p (kt s q) -> p kt s q", kt=2, s=2)
                        mk3 = mk_sb[:].rearrange("p (s q) -> p s q", s=2)
                        for ktl in range(2):
                            j = 2 * g + ktl - 4 * qb
                            if j >= 0:
                                nc.vector.tensor_mul(
                                    pt4[:, ktl, :, 128 * j : 128 * j + 128],
                                    pt4[:, ktl, :, 128 * j : 128 * j + 128],
                                    mk3,
                                )
                        pump(3)
                        if prev_pv is not None:
                            prev_pv()

                        def pv(g=g, pt=pt):
                            for ktl in range(2):
                                kt = 2 * g + ktl
                                j = kt - 4 * qb
                                roff = 128 * j if j >= 0 else 0
                                for s in range(2):
                                    h = 2 * hp + s
                                    nc.tensor.matmul(
                                        po[0:65, s * 512 + roff : s * 512 + 512],
                                        v_sb[:, kt, h * 65 : h * 65 + 65],
                                        pt[:, ktl * 1024 + s * 512 + roff : ktl * 1024 + (s + 1) * 512],
                                        start=(kt == 0), stop=(kt == n_kt - 1),
                                    )
                        prev_pv = pv
                    pump(2)
                    prev_pv()
                    # O^T + denominator row out of PSUM, then normalize this
                    # head-pair (one fast reciprocal serves both slots)
                    psq = {}
                    for s in range(2):
                        ps_sb = psn.tile([65, 512], F32, tag="ps",
                                         name=f"ps{hp}{s}")
                        nc.scalar.copy(
                            ps_sb[:], po[0:65, s * 512 : s * 512 + 512])
                        # DMA may read/write any partition row (engines cannot)
                        nc.gpsimd.dma_start(out=den[hp][s : s + 1, qb, :],
                                            in_=ps_sb[64:65, :])
                        psq[s] = ps_sb
                    nc.vector.reciprocal_approx_fast(rec[hp][:, qb, :],
                                                     den[hp][:, qb, :])
                    for s in range(2):
                        rc = nsm.tile([1, 512], F32, tag="rc", name="rc")
                        rb = nsm.tile([64, 512], F32, tag="rb", name="rb")
                        nc.gpsimd.dma_start(out=rc[:], in_=rec[hp][s : s + 1, qb, :])
                        nc.gpsimd.partition_broadcast(rb[:], rc[:])
                        nc.vector.tensor_mul(
                            ot_sb[64 * s : 64 * s + 64, hp, qs],
                            psq[s][0:64, :],
                            rb[:],
                        )

            # ---- main pipeline ----------------------------------------
            for _ in gen_qk(0):
                pass
            for _ in gen_v(0):
                pass
            gq = {}
            for qb in range(NQB):
                if qb < NQB - 1:
                    gq[qb + 1] = gen_qk(qb + 1)
                    fillers.append(gq[qb + 1])
                    fillers.append(gen_v(qb + 1))
                if qb >= 1:
                    fillers.append(gen_proj(qb - 1))
                emit_attn(qb)
                # attn(qb+1) S-matmuls read qt/kt of t-block qb+1, so those
                # must be fully emitted first; V is only read by late P@V
                # groups and can keep riding as filler.
                if qb < NQB - 1:
                    drain(gq[qb + 1])
            for _ in gen_proj(NQB - 1):
                pass
            while fillers:
                pump(1)

    nc.compile()
    return nc


_NC_CACHE = None


def _get_program():
    global _NC_CACHE
    if _NC_CACHE is None:
        _NC_CACHE = build_program()
    return _NC_CACHE


def make_in_maps(x, Wq, Wk, Wv, Wp):
    import ml_dtypes
    x = np.asarray(x, np.float32)
    Wq = np.asarray(Wq, np.float32)
    Wk = np.asarray(Wk, np.float32)
    Wv = np.asarray(Wv, np.float32)
    Wp = np.asarray(Wp, np.float32)
    tri = np.triu(np.ones((128, 128), np.float32))  # mask[k,q] = (k<=q)
    maskd = np.concatenate([tri, tri], axis=1).astype(ml_dtypes.bfloat16)
    in_maps = []
    for core in range(NCORES):
        b, hg = core // 4, core % 4
        sel = slice(hg * DSEL, (hg + 1) * DSEL)
        # SBUF images: [128, cc, ...] with partition index innermost in
        # the original feature dim (feature c -> (cc, p))
        # [128, cc, T] -> [128, tb, cc, 512] (contiguous per t-block)
        xi = x[b].T.reshape(NCC, 128, T).transpose(1, 0, 2)
        xi = xi.reshape(128, NCC, NQB, 512).transpose(0, 2, 1, 3)
        wqi = Wq[sel, :].T.reshape(NCC, 128, DSEL).transpose(1, 0, 2).reshape(128, NCC * DSEL)
        wki = Wk[sel, :].T.reshape(NCC, 128, DSEL).transpose(1, 0, 2).reshape(128, NCC * DSEL)
        wvi = Wv[sel, :].T.reshape(NCC, 128, DSEL).transpose(1, 0, 2).reshape(128, NCC * DSEL)
        wpi = Wp[:, sel].T.reshape(2, 128, C).transpose(1, 0, 2).reshape(128, 2 * C)
        in_maps.append({
            "xT": np.ascontiguousarray(xi.astype(ml_dtypes.bfloat16)),
            "wqT": np.ascontiguousarray(wqi.astype(ml_dtypes.bfloat16)),
            "wkT": np.ascontiguousarray(wki.astype(ml_dtypes.bfloat16)),
            "wvT": np.ascontiguousarray(wvi.astype(ml_dtypes.bfloat16)),
            "wpT": np.ascontiguousarray(wpi.astype(ml_dtypes.bfloat16)),
            "maskd": maskd,
        })
    return in_maps


def combine_outputs(results, bp):
    parts = [np.asarray(results[i]["out_p"], np.float32) for i in range(NCORES)]
    out = np.stack([
        parts[0] + parts[1] + parts[2] + parts[3],
        parts[4] + parts[5] + parts[6] + parts[7],
    ])
    return (out + np.asarray(bp, np.float32)).astype(np.float32)


def kernel(x, Wq, Wk, Wv, Wp, bp):
    nc = _get_program()
    in_maps = make_in_maps(x, Wq, Wk, Wv, Wp)
    res = bass_utils.run_bass_kernel_spmd(nc, in_maps, core_ids=list(range(NCORES)))
    return combine_outputs(res.results, bp)


# revision 16
# speedup vs baseline: 1.2130x; 1.2130x over previous
# Bass/Tile Trainium2 kernel for batched multi-head causal self-attention.
#
# Problem: x[B=2,T=2048,C=1024], 16 heads (hd=64), causal softmax attention,
# output projection. Full (unsharded) inputs in, full output out.
#
# Sharding (Megatron-style): 8 cores = 2 batch groups x 4 head groups.
# Core i handles batch b = i // 4 and heads [4*(i%4) : 4*(i%4)+4).
# Each core computes Q/K/V projections for its 4 heads, causal attention,
# and a partial output projection (contribution of its heads).  The host
# sums the 4 partials per batch (the Megatron all-reduce) and adds bias.
#
# Schedule: a software pipeline interleaved at ~0.5us granularity.
# Attention is ACT(exp)-bound, so QKV matmuls for t-block tb+1 and the
# output-projection matmuls for q-block qb-1 are emitted as "filler"
# chunks between the S and P@V matmuls of q-block qb, keeping the PE
# busy while the Scalar engine chews through exp.  All DMA is routed on
# the Sync/GpSimd/Vector queues so the Scalar queue runs ACTIVATE only.
#
# On-device layout notes:
#   - Everything is kept "transposed" (feature dim on partitions):
#     xT [C, T], QT/KT [64, T] per head.  Heads come in pairs packed on
#     the 128 partitions (even head at [0:64], odd head at [64:128]); the
#     K=64 S^T matmuls of a pair use explicit tile_position row groups so
#     they run concurrently on disjoint PE quadrants.
#   - V is stored [T, 64] per head augmented with a ones column (V') so
#     the P@V matmul also produces the softmax denominator (row 64).
#   - Softmax runs without max-subtraction (scores are bounded ~|10|, exp
#     is safe in fp32), so no partition-dim reductions are ever needed.
#   - S^T tiles for a (head-pair, 2 k-tiles) group live in one 4-bank
#     PSUM tile [128, 2048] = (ktile, slot, q); exp is ONE activation
#     instruction over the whole group.
#   - Causal masking: k-tiles strictly above the diagonal are skipped;
#     tiles crossing the diagonal get a triangular mask multiply (one
#     [128,2,128] DVE op per k-tile covering both slots) and a
#     column-restricted P@V matmul.
#   - QKV+attention internals are bf16; the projection also runs in bf16
#     (O^T normalized into bf16, Wp bf16) and partials are DMA'd out in
#     bf16; the host accumulates in fp32.

import numpy as np
from collections import deque

import concourse.bass as bass
import concourse.tile as tile
from concourse import bacc, mybir
from concourse import bass_utils

F32 = mybir.dt.float32
BF16 = mybir.dt.bfloat16
ATT_DT = BF16

B, T, C, H = 2, 2048, 1024, 16
HD = C // H            # 64 head dim
NCORES = 8
HPC = 4                # heads per core
DSEL = HPC * HD        # 256 feature dims per core
NTT = T // 128         # 16 t-tiles of 128
NCC = C // 128         # 8 c-chunks of 128
NQB = T // 512         # 4 q-blocks of 512


def build_program():
    nc = bacc.Bacc("TRN2", target_bir_lowering=False, debug=False)

    xT = nc.dram_tensor("xT", [128, NQB, NCC, 512], BF16, kind="ExternalInput").ap()
    wqT = nc.dram_tensor("wqT", [128, NCC * DSEL], BF16, kind="ExternalInput").ap()
    wkT = nc.dram_tensor("wkT", [128, NCC * DSEL], BF16, kind="ExternalInput").ap()
    wvT = nc.dram_tensor("wvT", [128, NCC * DSEL], BF16, kind="ExternalInput").ap()
    wpT = nc.dram_tensor("wpT", [128, 2 * C], BF16, kind="ExternalInput").ap()
    maskd = nc.dram_tensor("maskd", [128, 256], ATT_DT, kind="ExternalInput").ap()
    out_p = nc.dram_tensor("out_p", [T, C], BF16, kind="ExternalOutput").ap()

    scale = 1.0 / float(np.sqrt(HD))

    with tile.TileContext(nc) as tc:
        with (
            tc.tile_pool(name="consts", bufs=1) as consts,
            tc.tile_pool(name="persist", bufs=1) as persist,
            tc.tile_pool(name="pt", bufs=3) as ptpool,
            tc.tile_pool(name="psn", bufs=6) as psn,
            tc.tile_pool(name="nsm", bufs=4) as nsm,
            tc.tile_pool(name="outst", bufs=4) as outst,
            tc.tile_pool(name="sgp", bufs=1, space="PSUM") as sgp,
            tc.tile_pool(name="pop", bufs=1, space="PSUM") as pop,
            tc.tile_pool(name="gpp", bufs=2, space="PSUM") as gpp,
        ):
            # ---- ACT exp table pre-warm (one-time ~2.7us table DMA) ----
            wrm_in = consts.tile([1, 16], F32, tag="wrm_in")
            wrm_out = consts.tile([1, 16], BF16, tag="wrm_out")
            nc.vector.memset(wrm_in[:], 0.0)
            nc.scalar.activation(
                out=wrm_out[:], in_=wrm_in[:],
                func=mybir.ActivationFunctionType.Exp, scale=1.0,
            )

            # ---- constants / weights ----------------------------------
            wq_sb = consts.tile([128, NCC, DSEL], BF16, tag="wq")
            wk_sb = consts.tile([128, NCC, DSEL], BF16, tag="wk")
            wv_sb = consts.tile([128, NCC, DSEL], BF16, tag="wv")
            wp_sb = consts.tile([128, 2, C], BF16, tag="wp")
            mk_sb = consts.tile([128, 256], ATT_DT, tag="mk")
            nc.sync.dma_start(out=mk_sb[:], in_=maskd)

            # persistent activations: QT/KT/OT head pairs packed on
            # partitions ([0:64] even slot, [64:128] odd slot), free = t
            qt_sb = persist.tile([128, 2, T], ATT_DT, tag="qt")
            kt_sb = persist.tile([128, 2, T], ATT_DT, tag="kt")
            ot_sb = persist.tile([128, 2, T], ATT_DT, tag="ot")
            # V' per k-tile: 4 heads x (64 V cols + 1 ones col)
            v_sb = persist.tile([128, NTT, HPC * (HD + 1)], ATT_DT, tag="v")
            xt_all = persist.tile([128, NQB, NCC, 512], BF16, tag="xt")
            den = [persist.tile([2, NQB, 512], F32, tag=f"den{hp}", name=f"den{hp}")
                   for hp in range(2)]
            rec = [persist.tile([2, NQB, 512], F32, tag=f"rec{hp}", name=f"rec{hp}")
                   for hp in range(2)]

            ones_sb = consts.tile([128, NTT], F32, tag="ones")
            nc.vector.memset(ones_sb[:], 1.0)
            for h in range(HPC):
                nc.vector.tensor_copy(
                    out=v_sb[:, :, h * 65 + 64 : h * 65 + 65],
                    in_=ones_sb[:].rearrange("p (t o) -> p t o", o=1),
                )

            # ---- input DMAs: few, large, queue-parallel ---------------
            # x is staged in DRAM as [128, tb, cc, 512] so each t-block is
            # one contiguous 8KB-per-partition DMA; t-blocks 1-3 ride the
            # Scalar queue (idle until attention starts).
            nc.sync.dma_start(
                out=wq_sb[:].rearrange("p c d -> p (c d)"), in_=wqT[:])
            nc.sync.dma_start(
                out=xt_all[:, 0].rearrange("p c t -> p (c t)"),
                in_=xT[:, 0].rearrange("p c t -> p (c t)"))
            nc.sync.dma_start(
                out=wk_sb[:].rearrange("p c d -> p (c d)"), in_=wkT[:])
            nc.sync.dma_start(
                out=wv_sb[:].rearrange("p c d -> p (c d)"), in_=wvT[:])
            for tb in range(1, 4):
                nc.scalar.dma_start(
                    out=xt_all[:, tb].rearrange("p c t -> p (c t)"),
                    in_=xT[:, tb].rearrange("p c t -> p (c t)"))
            nc.sync.dma_start(
                out=wp_sb[:].rearrange("p h c -> p (h c)"), in_=wpT[:])

            # ---- filler generators (PE work interleaved into attention)
            def gen_qk(tb):
                ts = slice(tb * 512, tb * 512 + 512)
                # Q then K: one 512-wide matmul chain per head-pair
                for wsb, dst in ((wq_sb, qt_sb), (wk_sb, kt_sb)):
                    for pr in range(2):
                        acc = gpp.tile([128, 512], F32, tag="gp", name="acc")
                        for cc in range(NCC):
                            nc.tensor.matmul(
                                acc[:],
                                wsb[:, cc, pr * 128 : pr * 128 + 128],
                                xt_all[:, tb, cc, :],
                                start=(cc == 0), stop=(cc == NCC - 1),
                            )
                            if cc % 3 == 2:
                                yield
                        nc.vector.tensor_copy(out=dst[:, pr, ts], in_=acc[:])
                        yield

            def gen_v(tb):
                # V: [t, d] layout, two 256-t halves of the block
                for half in range(2):
                    accv = gpp.tile([128, 512], F32, tag="gp", name="accv")
                    for cc in range(NCC):
                        for tt in range(2):
                            tl = half * 256 + tt * 128
                            nc.tensor.matmul(
                                accv[:, tt * 256 : tt * 256 + 256],
                                xt_all[:, tb, cc, tl : tl + 128],
                                wv_sb[:, cc, :],
                                start=(cc == 0 and tt == 0),
                                stop=(cc == NCC - 1 and tt == 1),
                            )
                        if cc % 3 == 2:
                            yield
                    t4 = tb * 4 + half * 2
                    pv3 = accv[:].rearrange("p (tt d) -> p tt d", tt=2)
                    for h in range(HPC):
                        nc.vector.tensor_copy(
                            out=v_sb[:, t4 : t4 + 2, h * 65 : h * 65 + 64],
                            in_=pv3[:, :, h * 64 : h * 64 + 64],
                        )
                    yield

            def gen_proj(qb):
                for tt in range(4 * qb, 4 * qb + 4):
                    tloc = slice(tt * 128, tt * 128 + 128)
                    pc0 = gpp.tile([128, 512], F32, tag="gp", name="pc0")
                    pc1 = gpp.tile([128, 512], F32, tag="gp", name="pc1")
                    for hpp in range(2):
                        for cb, pc in enumerate((pc0, pc1)):
                            nc.tensor.matmul(
                                pc[:],
                                ot_sb[:, hpp, tloc],
                                wp_sb[:, hpp, cb * 512 : cb * 512 + 512],
                                start=(hpp == 0), stop=(hpp == 1),
                            )
                        yield
                    ob = outst.tile([128, 1024], BF16, tag="ob", name="ob")
                    nc.vector.tensor_copy(out=ob[:, 0:512], in_=pc0[:])
                    nc.vector.tensor_copy(out=ob[:, 512:1024], in_=pc1[:])
                    nc.sync.dma_start(
                        out=out_p[tt * 128 : tt * 128 + 128, :], in_=ob[:])
                    yield

            fillers = deque()

            def pump(n):
                while n > 0 and fillers:
                    try:
                        next(fillers[0])
                        n -= 1
                    except StopIteration:
                        fillers.popleft()

            def drain(gen):
                while gen in fillers:
                    pump(1)

            # ---- attention per (q-block, head-pair) -------------------
            def emit_attn(qb):
                qs = slice(qb * 512, qb * 512 + 512)
                n_kt = 4 * (qb + 1)
                for hp in range(2):
                    po = pop.tile([128, 1024], F32, tag="po", name="po")
                    prev_pv = None
                    for g in range(n_kt // 2):
                        sg = sgp.tile([128, 2048], F32, tag="sg", name="sg")
                        pt = ptpool.tile([128, 2048], ATT_DT, tag="pt", name="pt")
                        # S^T: 4 matmuls, row-split pairs run concurrently
                        for ktl in range(2):
                            kt = 2 * g + ktl
                            for s in range(2):
                                psl = slice(64 * s, 64 * s + 64)
                                nc.tensor.matmul(
                                    sg[:, ktl * 1024 + s * 512 : ktl * 1024 + s * 512 + 512],
                                    kt_sb[psl, hp, kt * 128 : kt * 128 + 128],
                                    qt_sb[psl, hp, qs],
                                    start=True, stop=True,
                                    tile_position=(64 * s, 0),
                                )
                        # exp over the whole group (both k-tiles, both slots)
                        nc.scalar.activation(
                            out=pt[:], in_=sg[:],
                            func=mybir.ActivationFunctionType.Exp,
                            scale=scale,
                        )
                        # triangular mask on diagonal blocks (both slots/op)
                        pt4 = pt[:].rearrange("p (kt s q) -> p kt s q", kt=2, s=2)
                        mk3 = mk_sb[:].rearrange("p (s q) -> p s q", s=2)
                        for ktl in range(2):
                            j = 2 * g + ktl - 4 * qb
                            if j >= 0:
                                nc.vector.tensor_mul(
                                    pt4[:, ktl, :, 128 * j : 128 * j + 128],
                                    pt4[:, ktl, :, 128 * j : 128 * j + 128],
                                    mk3,
                                )
                        pump(3)
                        if prev_pv is not None:
                            prev_pv()

                        def pv(g=g, pt=pt):
                            for ktl in range(2):
                                kt = 2 * g + ktl
                                j = kt - 4 * qb
                                roff = 128 * j if j >= 0 else 0
                                for s in range(2):
                                    h = 2 * hp + s
                                    nc.tensor.matmul(
                                        po[0:65, s * 512 + roff : s * 512 + 512],
                                        v_sb[:, kt, h * 65 : h * 65 + 65],
                                        pt[:, ktl * 1024 + s * 512 + roff : ktl * 1024 + (s + 1) * 512],
                                        start=(kt == 0), stop=(kt == n_kt - 1),
                                    )
                        prev_pv = pv
                    pump(2)
                    prev_pv()
                    # O^T + denominator row out of PSUM, then normalize this
                    # head-pair (one fast reciprocal serves both slots)
                    psq = {}
                    for s in range(2):
                        ps_sb = psn.tile([65, 512], F32, tag="ps",
                                         name=f"ps{hp}{s}")
                        nc.vector.tensor_copy(
                            out=ps_sb[:], in_=po[0:65, s * 512 : s * 512 + 512])
                        # DMA may read/write any partition row (engines cannot)
                        nc.gpsimd.dma_start(out=den[hp][s : s + 1, qb, :],
                                            in_=ps_sb[64:65, :])
                        psq[s] = ps_sb
                    nc.vector.reciprocal_approx_fast(rec[hp][:, qb, :],
                                                     den[hp][:, qb, :])
                    for s in range(2):
                        rc = nsm.tile([1, 512], F32, tag="rc", name="rc")
                        rb = nsm.tile([64, 512], F32, tag="rb", name="rb")
                        nc.gpsimd.dma_start(out=rc[:], in_=rec[hp][s : s + 1, qb, :])
                        nc.gpsimd.partition_broadcast(rb[:], rc[:])
                        nc.vector.tensor_mul(
                            ot_sb[64 * s : 64 * s + 64, hp, qs],
                            psq[s][0:64, :],
                            rb[:],
                        )

            # ---- main pipeline ----------------------------------------
            for _ in gen_qk(0):
                pass
            for _ in gen_v(0):
                pass
            gq = {}
            for qb in range(NQB):
                if qb < NQB - 1:
                    gq[qb + 1] = gen_qk(qb + 1)
                    fillers.append(gq[qb + 1])
                    fillers.append(gen_v(qb + 1))
                else:
                    # attn(3) is the longest ACT-bound stretch and has no
                    # QKV left to interleave: feed it all three finished
                    # projection blocks to keep the PE warm
                    for pq_ in range(NQB - 1):
                        fillers.append(gen_proj(pq_))
                emit_attn(qb)
                # attn(qb+1) S-matmuls read qt/kt of t-block qb+1, so those
                # must be fully emitted first; V is only read by late P@V
                # groups and can keep riding as filler.
                if qb < NQB - 1:
                    drain(gq[qb + 1])
            for _ in gen_proj(NQB - 1):
                pass
            while fillers:
                pump(1)

    nc.compile()
    return nc


_NC_CACHE = None


def _get_program():
    global _NC_CACHE
    if _NC_CACHE is None:
        _NC_CACHE = build_program()
    return _NC_CACHE


def make_in_maps(x, Wq, Wk, Wv, Wp):
    import ml_dtypes
    x = np.asarray(x, np.float32)
    Wq = np.asarray(Wq, np.float32)
    Wk = np.asarray(Wk, np.float32)
    Wv = np.asarray(Wv, np.float32)
    Wp = np.asarray(Wp, np.float32)
    tri = np.triu(np.ones((128, 128), np.float32))  # mask[k,q] = (k<=q)
    maskd = np.concatenate([tri, tri], axis=1).astype(ml_dtypes.bfloat16)
    in_maps = []
    for core in range(NCORES):
        b, hg = core // 4, core % 4
        sel = slice(hg * DSEL, (hg + 1) * DSEL)
        # SBUF images: [128, cc, ...] with partition index innermost in
        # the original feature dim (feature c -> (cc, p))
        # [128, cc, T] -> [128, tb, cc, 512] (contiguous per t-block)
        xi = x[b].T.reshape(NCC, 128, T).transpose(1, 0, 2)
        xi = xi.reshape(128, NCC, NQB, 512).transpose(0, 2, 1, 3)
        wqi = Wq[sel, :].T.reshape(NCC, 128, DSEL).transpose(1, 0, 2).reshape(128, NCC * DSEL)
        wki = Wk[sel, :].T.reshape(NCC, 128, DSEL).transpose(1, 0, 2).reshape(128, NCC * DSEL)
        wvi = Wv[sel, :].T.reshape(NCC, 128, DSEL).transpose(1, 0, 2).reshape(128, NCC * DSEL)
        wpi = Wp[:, sel].T.reshape(2, 128, C).transpose(1, 0, 2).reshape(128, 2 * C)
        in_maps.append({
            "xT": np.ascontiguousarray(xi.astype(ml_dtypes.bfloat16)),
            "wqT": np.ascontiguousarray(wqi.astype(ml_dtypes.bfloat16)),
            "wkT": np.ascontiguousarray(wki.astype(ml_dtypes.bfloat16)),
            "wvT": np.ascontiguousarray(wvi.astype(ml_dtypes.bfloat16)),
            "wpT": np.ascontiguousarray(wpi.astype(ml_dtypes.bfloat16)),
            "maskd": maskd,
        })
    return in_maps


def combine_outputs(results, bp):
    parts = [np.asarray(results[i]["out_p"], np.float32) for i in range(NCORES)]
    out = np.stack([
        parts[0] + parts[1] + parts[2] + parts[3],
        parts[4] + parts[5] + parts[6] + parts[7],
    ])
    return (out + np.asarray(bp, np.float32)).astype(np.float32)


def kernel(x, Wq, Wk, Wv, Wp, bp):
    nc = _get_program()
    in_maps = make_in_maps(x, Wq, Wk, Wv, Wp)
    res = bass_utils.run_bass_kernel_spmd(nc, in_maps, core_ids=list(range(NCORES)))
    return combine_outputs(res.results, bp)
